# revision 3
# baseline (speedup 1.0000x reference)
"""Self-contained TRN2 Bass kernel for the RGCN message-passing problem.

kernel(**inputs) takes the FULL unsharded inputs (text, src, dst, rel,
bases, comp, bias), shards edges by destination window across the 8
NeuronCores, runs the SPMD Bass program via run_bass_kernel_spmd, and
returns the full [64, 512, 256] float32 output.

v2: the per-edge one-hot weight matrix (W1h) is built on-chip from an
8 B/edge (dstloc, comp[rel]) stream instead of streamed dense from HBM
(384 B/edge), and stage-2 applies the bases as the stationary matmul
operand over batches of 8 windows (N=512), with bias+ReLU fused into
the scalar-engine activation and a transposed [O, dcore] output that
the host de-transposes.
"""

import numpy as np
import ml_dtypes

import concourse.bass as bass
import concourse.tile as tile
from concourse import bacc, mybir

F = 256      # in features
O = 256      # out features
NB = 3       # bases
W = 64       # dst rows per window
GROUP = 8    # windows per stage-2 matmul group (N = GROUP*W = 512)
PBW = 2      # windows per stage-1 PSUM bank (PBW*NB*W*4B <= 2 KiB)
CPC = 8      # chunks per gather call
GBUFS = 6    # gather tile buffering depth
NQ = 4       # SWDGE queues (ucode max)


def plan_calls(slot_cws, cpc):
    """Split each window slot into gather calls of <= cpc chunks.
    Returns list of (slot, chunk_lo, n_chunks) in execution order."""
    calls = []
    for i, cw in enumerate(slot_cws):
        lo = 0
        while lo < cw:
            n = min(cpc, cw - lo)
            calls.append((i, lo, n))
            lo += n
    return calls


def build_program(n_nodes, slot_cws, cpc=CPC, n_cores=8):
    slot_cws = list(slot_cws)
    nw = len(slot_cws)
    assert nw % GROUP == 0 and GROUP % PBW == 0
    nchunks = sum(slot_cws)
    epad = nchunks * 128
    dcore = nw * W
    calls = plan_calls(slot_cws, cpc)

    bf16 = mybir.dt.bfloat16
    f32 = mybir.dt.float32
    i16 = mybir.dt.int16

    # bf16 DRAM I/O breaks NEFF load under the PJRT path; all bf16 payloads
    # travel as int16 containers and are bitcast on-chip.
    nc = bacc.Bacc("TRN2", target_bir_lowering=False, debug=False,
                   num_devices=n_cores, num_swdge_queues=NQ)
    h_d = nc.dram_tensor("h", [n_nodes, F], i16, kind="ExternalInput").ap()
    gidx_d = nc.dram_tensor("gidx", [128, epad // 16], i16,
                            kind="ExternalInput").ap()
    dloc_d = nc.dram_tensor("dloc", [128, nchunks], i16,
                            kind="ExternalInput").ap()
    w3_d = nc.dram_tensor("w3", [128, NB, nchunks], i16,
                          kind="ExternalInput").ap()
    iot_d = nc.dram_tensor("iot", [128, W, cpc], i16,
                           kind="ExternalInput").ap()
    bases_d = nc.dram_tensor("bases", [NB, F, O], i16,
                             kind="ExternalInput").ap()
    bias_d = nc.dram_tensor("bias", [128, 2], f32, kind="ExternalInput").ap()
    out_d = nc.dram_tensor("out", [O, dcore], i16, kind="ExternalOutput").ap()

    relu = mybir.ActivationFunctionType.Relu
    eq = mybir.AluOpType.is_equal
    mult = mybir.AluOpType.mult

    with tile.TileContext(nc) as tc:
        with (
            tc.tile_pool(name="const", bufs=1) as cpool,
            tc.tile_pool(name="gather", bufs=GBUFS) as gpool,
            tc.tile_pool(name="oh", bufs=3) as ohpool,
            tc.tile_pool(name="wt", bufs=4) as wpool,
            tc.tile_pool(name="abt", bufs=2) as apool,
            tc.tile_pool(name="ost", bufs=2) as opool,
            tc.tile_pool(name="ps1", bufs=2, space="PSUM") as ps1,
            tc.tile_pool(name="ps2", bufs=2, space="PSUM") as ps2,
        ):
            # ---- prologue ----
            gidx_sb = cpool.tile([128, epad // 16], i16)
            # call 0's slice first so the gather pipeline starts immediately
            ntot = epad // 16
            cuts = [0, calls[0][2] * 8]
            cuts += [cuts[1] + (ntot - cuts[1]) * k // 3 for k in (1, 2, 3)]
            for lo, hi in zip(cuts[:-1], cuts[1:]):
                if hi > lo:
                    nc.sync.dma_start(gidx_sb[:, lo:hi], gidx_d[:, lo:hi])
            # W-build constants go on the scalar HWDGE ring so they land in
            # parallel with gidx on the sync ring
            dloc_sb = cpool.tile([128, nchunks], i16)
            nc.scalar.dma_start(dloc_sb[:], dloc_d[:])
            w3_sb = cpool.tile([128, NB, nchunks], i16)
            nc.scalar.dma_start(w3_sb[:], w3_d[:])
            iot_sb = cpool.tile([128, W, cpc], i16)
            nc.scalar.dma_start(iot_sb[:], iot_d[:])
            bases_i = cpool.tile([128, NB, 2, O], i16)
            for b in range(NB):
                for h in range(2):
                    nc.scalar.dma_start(bases_i[:, b, h, :],
                                        bases_d[b, h * 128:(h + 1) * 128, :])
            bias_sb = cpool.tile([128, 2], f32)
            nc.scalar.dma_start(bias_sb[:], bias_d[:])

            # ---- main pipeline ----
            p1 = None
            abt = None
            chunk_base = 0
            for j, (slot, clo, ncall) in enumerate(calls):
                nidx = ncall * 128
                G = gpool.tile([128, cpc, F], i16, tag="G", name="G")
                # pads carry index 0 (a real row; the on-chip W tile zeroes
                # their weights), so every gathered row is valid
                nc.gpsimd.dma_gather(
                    G[:, 0:ncall, :], h_d[:],
                    gidx_sb[:, chunk_base * 8:(chunk_base + ncall) * 8],
                    nidx, nidx, F, queue_num=j % NQ)
                # on-chip W build: oh[p,d,c] = (iota d == dstloc[p,c]),
                # Wt[p,b,d,c] = oh[p,d,c] * w3[p,b,c]; chunk-last layouts
                # keep every DVE operand dense in its final axis (2x mode)
                oh = ohpool.tile([128, W, cpc], bf16, tag="oh", name="oh")
                nc.vector.tensor_tensor(
                    oh[:, :, 0:ncall],
                    dloc_sb[:, chunk_base:chunk_base + ncall]
                        .unsqueeze(1).broadcast_to([128, W, ncall]),
                    iot_sb[:, :, 0:ncall],
                    eq)
                Wt = wpool.tile([128, NB, W, cpc], bf16, tag="Wt", name="Wt")
                nc.vector.tensor_tensor(
                    Wt[:, :, :, 0:ncall],
                    oh[:, :, 0:ncall]
                        .unsqueeze(1).broadcast_to([128, NB, W, ncall]),
                    w3_sb[:, :, chunk_base:chunk_base + ncall].bitcast(bf16)
                        .unsqueeze(2).broadcast_to([128, NB, W, ncall]),
                    mult)
                for c in range(ncall):
                    cw = clo + c
                    if cw == 0 and slot % PBW == 0:
                        p1 = [ps1.tile([128, PBW, NB, W], f32,
                                       tag=f"p1h{h}", name=f"p1h{h}")
                              for h in range(2)]
                    last = (cw == slot_cws[slot] - 1)
                    for h in range(2):
                        nc.tensor.matmul(
                            p1[h][:, slot % PBW, :, :],
                            G[:, c, h * 128:(h + 1) * 128].bitcast(bf16),
                            Wt[:, :, :, c],
                            start=(cw == 0), stop=last)
                    if last and slot % PBW == PBW - 1:
                        k2 = (slot % GROUP) // PBW
                        if k2 == 0:
                            abt = apool.tile([128, 2, NB, GROUP, W], bf16,
                                             tag="abt", name="abt")
                        for h in range(2):
                            eng = nc.vector if h == 0 else nc.scalar
                            src = p1[h][:, :, :, :].rearrange(
                                "p w b d -> p b w d")
                            dst = abt[:, h, :, k2 * PBW:(k2 + 1) * PBW, :]
                            if h == 0:
                                eng.tensor_copy(dst, src)
                            else:
                                eng.copy(dst, src)
                        if k2 == GROUP // PBW - 1:
                            g = slot // GROUP
                            for o in range(2):
                                p2 = ps2.tile([128, GROUP * W], f32,
                                              tag=f"p2o{o}", name=f"p2o{o}")
                                k = 0
                                for h in range(2):
                                    for b in range(NB):
                                        nc.tensor.matmul(
                                            p2[:],
                                            bases_i[:, b, h,
                                                    o * 128:(o + 1) * 128]
                                                .bitcast(bf16),
                                            abt[:, h, b, :, :],
                                            start=(k == 0),
                                            stop=(k == 2 * NB - 1))
                                        k += 1
                                osb = opool.tile([128, GROUP * W], bf16,
                                                 tag=f"osb{o}",
                                                 name=f"osb{o}")
                                nc.scalar.activation(
                                    osb[:], p2[:], relu,
                                    bias=bias_sb[:, o:o + 1], scale=1.0)
                                nc.sync.dma_start(
                                    out_d[o * 128:(o + 1) * 128,
                                          g * GROUP * W:(g + 1) * GROUP * W],
                                    osb[:].bitcast(i16))
                chunk_base += ncall

    nc.compile()
    return nc


def host_prep(src, dst, rel, comp, n_nodes, n_cores, cpc=CPC):
    """Sort/deal/pad edges; build gather indices and per-edge (dstloc, w)."""
    dcore = n_nodes // n_cores
    nw = dcore // W
    ngw = n_cores * nw
    w_edge = comp[rel].astype(ml_dtypes.bfloat16)        # [E, NB]
    gw = (dst // W).astype(np.int64)
    order = np.argsort(gw, kind="stable")
    counts = np.bincount(gw, minlength=ngw)
    starts = np.concatenate([[0], np.cumsum(counts)])

    # deal windows to cores by descending count; slot capacity = group max
    ranked = np.argsort(-counts, kind="stable")
    slot_cws = [max(1, -(-int(counts[ranked[n_cores * i]]) // 128))
                for i in range(nw)]
    nchunks = sum(slot_cws)
    epad = nchunks * 128

    gidx = np.zeros((n_cores, epad), np.int16)
    dloc_a = np.zeros((n_cores, 128, nchunks), np.int16)
    w3_a = np.zeros((n_cores, 128, NB, nchunks), ml_dtypes.bfloat16)
    win_of_slot = np.zeros((n_cores, nw), np.int64)
    dstloc = (dst % W).astype(np.int64)

    slot_base = np.zeros(nw, np.int64)
    acc = 0
    for i, cw in enumerate(slot_cws):
        slot_base[i] = acc
        acc += cw
    for k in range(n_cores):
        for i in range(nw):
            wid = int(ranked[n_cores * i + k])
            win_of_slot[k, i] = wid
            es = order[starts[wid]:starts[wid + 1]]
            base = slot_base[i] * 128
            n = len(es)
            pos = base + np.arange(n)
            gidx[k, pos] = src[es].astype(np.int16)
            dloc_a[k, pos % 128, pos // 128] = dstloc[es].astype(np.int16)
            w3_a[k, pos % 128, :, pos // 128] = w_edge[es]

    # wrap gidx: idx i -> partition i%16, slot i//16; replicate to 128 parts
    gidx_w = gidx.reshape(n_cores, epad // 16, 16).transpose(0, 2, 1)
    gidx_w = np.tile(gidx_w, (1, 8, 1)).copy()
    iot = np.broadcast_to(
        np.arange(W, dtype=np.int16)[None, :, None], (128, W, cpc)).copy()
    return gidx_w, dloc_a, w3_a, iot, tuple(slot_cws), win_of_slot


def rgcn_kernel(text, src, dst, rel, bases, comp, bias, n_cores=8,
                run_fn=None, cpc=CPC, nc_cache={}):
    """Full-input kernel: shard, run on 8 cores, reassemble output."""
    Bt, St, INF = text.shape
    n_nodes = Bt * St
    h = text.reshape(n_nodes, INF)

    src = np.asarray(src).astype(np.int64)
    dst = np.asarray(dst).astype(np.int64)
    rel = np.asarray(rel).astype(np.int64)
    bases_np = np.asarray(bases, np.float32)
    comp_np = np.asarray(comp, np.float32)
    bias_np = np.asarray(bias, np.float32)

    gidx_w, dloc_a, w3_a, iot, slot_cws, win_of_slot = host_prep(
        src, dst, rel, comp_np, n_nodes, n_cores, cpc)
    key = (n_nodes, slot_cws, cpc, n_cores)
    if key not in nc_cache:
        nc_cache[key] = build_program(n_nodes, slot_cws, cpc, n_cores)
    nc = nc_cache[key]

    h_bf = np.asarray(h, np.float32).astype(ml_dtypes.bfloat16).view(np.int16)
    bases_bf = bases_np.astype(ml_dtypes.bfloat16).view(np.int16)
    bias_t = np.ascontiguousarray(
        bias_np.reshape(2, 128).T.astype(np.float32))

    in_maps = [
        dict(h=h_bf, gidx=gidx_w[k], dloc=dloc_a[k],
             w3=w3_a[k].view(np.int16), iot=iot,
             bases=bases_bf, bias=bias_t)
        for k in range(n_cores)
    ]
    from concourse.bass_utils import run_bass_kernel_spmd
    if run_fn is None:
        res = run_bass_kernel_spmd(nc, in_maps, list(range(n_cores)))
        outs = [res.results[k]["out"] for k in range(n_cores)]
    else:
        outs = run_fn(nc, in_maps)

    out = np.zeros((n_nodes, O), np.float32)
    nw = len(slot_cws)
    for k in range(n_cores):
        ok = outs[k].view(ml_dtypes.bfloat16).astype(np.float32)  # [O, dcore]
        for i in range(nw):
            wid = win_of_slot[k][i]
            out[wid * W:(wid + 1) * W] = ok[:, i * W:(i + 1) * W].T
    return out.reshape(Bt, St, O)


_NC_CACHE = {}


def kernel(text, src, dst, rel, bases, comp, bias):
    out = rgcn_kernel(
        np.asarray(text, np.float32),
        np.asarray(src), np.asarray(dst), np.asarray(rel),
        np.asarray(bases, np.float32), np.asarray(comp, np.float32),
        np.asarray(bias, np.float32),
        n_cores=8, nc_cache=_NC_CACHE)
    return np.ascontiguousarray(out, np.float32)
